# revision 13
# baseline (speedup 1.0000x reference)
"""AnyStory Flux attention processor on 8 TRN2 NeuronCores.

Sharding: tensor-parallel over heads (24 heads -> 3 per core), no
collectives; host gathers along the head axis.

v2 design (vs baseline): PE runs ONLY bf16 QK/PV matmuls (no mask
matmuls, no normalization reads). Everything else is moved off PE:
  - masks: DVE scalar_tensor_tensor pre-adds the (KEXP-scaled) additive
    mask into the QK PSUM tile, then ACT computes the real exp. The mask
    never touches PE.
  - exp: split ACT (real exp, scale=1/KEXP) / DVE (Schraudolph bf16
    fast-exp: int16 bits = round(y + BIAS + C), bitcast to bf16; ~±3%
    on its share of tiles, used only on unmasked k-tiles).
  - normalization: on the HOST. The device emits the unnormalized PV
    accumulator with a ones-column denominator (bf16), host divides.
Logits are computed pre-scaled by KEXP = 128/ln2 (folded into q on the
host) so the Schraudolph bit trick is a single DVE op and ACT just uses
scale=1/KEXP.

K extent per head is 3136 (txt+redux+img+ref) = 24 full k-tiles + one
64-row half tile; router keys are excluded by construction (seg1 never
attends to them), so no kill-mask is needed.
"""

import math
import numpy as np
import ml_dtypes
from contextlib import ExitStack

import concourse.bass as bass
import concourse.tile as tile
from concourse import mybir, bacc
from concourse.bass_utils import run_bass_kernel_spmd

# ---- problem constants (hardcoded; kernel.py must be self-contained)
B, H, D = 1, 24, 128
TXT, REDUX, IMG, REF, ROUTER, NCOND = 512, 64, 2048, 512, 32, 2
S = TXT + REDUX + IMG + REF + ROUTER          # 3168
TE = TXT                                       # 512
TRE = TE + REDUX                               # 576
TRI = TRE + IMG                                # 2624
TRIR = TRI + REF                               # 3136
REF_SHIFT = 1.5
HPC = H // 8                                   # heads per core = 3

KEXP = 128.0 * 1.4426950408889634              # 184.664 logit prescale
SBIAS = 16256.0                                # 127<<7 (bf16 exponent bias)
SCHC = -5.60                                   # Schraudolph centering (RN)
MASK_CLAMP = -30.0 * KEXP                      # keeps int16 bits positive

NT = 25                                        # 24 full k-tiles + 1 half
MASKED = {4, 20, 21, 22, 23, 24}
GROUPS = [(0, 1), (2, 3), (4, 5), (6, 7), (8, 9), (10, 11), (12, 13),
          (14, 15), (16, 17), (18, 19), (20, 21), (22, 23), (24,)]
DVE_GROUPS = {4}                               # unmasked groups on DVE fast-exp
QBLOCKS = [(0, 384), (384, 384), (768, 384), (1152, 384),
           (1536, 384), (1920, 384), (2304, 320)]

F32 = mybir.dt.float32
BF16 = mybir.dt.bfloat16
I16 = mybir.dt.int16
EXP = mybir.ActivationFunctionType.Exp
ADD = mybir.AluOpType.add
BYP = mybir.AluOpType.bypass


def _subs(qw):
    out, o = [], 0
    while o < qw:
        w = min(128, qw - o)
        out.append((o, w))
        o += w
    return out


def build_nc():
    nc = bacc.Bacc()
    qt_d = nc.declare_dram_parameter("qt", [HPC, 128, S], BF16, isOutput=False)
    kt_d = nc.declare_dram_parameter("kt", [HPC, 128, S], BF16, isOutput=False)
    v1_d = nc.declare_dram_parameter("v1", [HPC, 128, NT, 129], BF16, isOutput=False)
    v2_d = nc.declare_dram_parameter("v2", [HPC, 128, 4, 129], BF16, isOutput=False)
    v3_d = nc.declare_dram_parameter("v3", [HPC, 128, 17, 129], BF16, isOutput=False)
    # masks, KEXP-scaled, clamped, bf16; shared across heads
    m4_d = nc.declare_dram_parameter("m4", [128, 2, TRI], BF16, isOutput=False)
    mA_d = nc.declare_dram_parameter("mA", [128, 2, TRI], BF16, isOutput=False)
    mB_d = nc.declare_dram_parameter("mB", [128, 2, TRI], BF16, isOutput=False)
    m24_d = nc.declare_dram_parameter("m24", [64, TRI], BF16, isOutput=False)
    o1_d = nc.declare_dram_parameter("o1", [HPC, 7, 128, 387], BF16, isOutput=True)
    o2_d = nc.declare_dram_parameter("o2", [HPC, 2, 128, 258], BF16, isOutput=True)
    o3_d = nc.declare_dram_parameter("o3", [HPC, 32, 129], BF16, isOutput=True)

    with ExitStack() as ctx:
        tc = ctx.enter_context(tile.TileContext(nc))
        const = ctx.enter_context(tc.tile_pool(name="const", bufs=1))
        stp = ctx.enter_context(tc.tile_pool(name="st", bufs=3, space="PSUM"))
        accp = ctx.enter_context(tc.tile_pool(name="acc", bufs=2, space="PSUM"))
        ptp = ctx.enter_context(tc.tile_pool(name="pt", bufs=8))
        stgp = ctx.enter_context(tc.tile_pool(name="stg", bufs=4))

        # ---- persistent SBUF
        m4_sb = const.tile([128, 2, TRI], BF16, tag="m4")
        mA_sb = const.tile([128, 2, TRI], BF16, tag="mA")
        mB_sb = const.tile([128, 2, TRI], BF16, tag="mB")
        m24_sb = const.tile([64, TRI], BF16, tag="m24")
        kt_sb, qt_sb, v1_sb, v2_sb, v3_sb = [], [], [], [], []
        for h in range(HPC):
            kt = const.tile([128, S], BF16, tag=f"kt{h}")
            qt = const.tile([128, S], BF16, tag=f"qt{h}")
            v1 = const.tile([128, NT, 129], BF16, tag=f"v1{h}")
            v2 = const.tile([128, 4, 129], BF16, tag=f"v2{h}")
            v3 = const.tile([128, 17, 129], BF16, tag=f"v3{h}")
            kt_sb.append(kt); qt_sb.append(qt); v1_sb.append(v1)
            v2_sb.append(v2); v3_sb.append(v3)

            def ktc(t0, t1, h=h, kt=kt):
                nc.sync.dma_start(kt[:, t0:t1], kt_d[h, :, t0:t1])

            def qtc(c0, c1, h=h, qt=qt):
                nc.sync.dma_start(qt[:, c0:c1], qt_d[h, :, c0:c1])

            def v1c(t0, t1, h=h, v1=v1):
                nc.sync.dma_start(v1[:, t0:t1, :], v1_d[h, :, t0:t1, :])

            if h == 0:
                # just-in-time order: chunks land right before the pipeline
                # consumes them (QK needs kt+qt, group-2 stt needs m4 early,
                # PV needs v1 two items after its exp, groups 10-12 need
                # mA/mB/m24 at ~items 10-12).
                ktc(0, 256); qtc(0, 384); v1c(0, 2); ktc(256, 768)
                nc.sync.dma_start(m4_sb[:, :, 0:768], m4_d[:, :, 0:768])
                v1c(2, 6)
                ktc(768, 1536); v1c(6, 12)
                nc.sync.dma_start(m4_sb[:, :, 768:TRI], m4_d[:, :, 768:TRI])
                nc.sync.dma_start(mA_sb[:, :, 0:768], mA_d[:, :, 0:768])
                ktc(1536, 2304); v1c(12, 18)
                nc.sync.dma_start(mB_sb[:, :, 0:768], mB_d[:, :, 0:768])
                nc.sync.dma_start(m24_sb[:, 0:768], m24_d[:, 0:768])
                ktc(2304, S); v1c(18, NT)
                qtc(384, 1152)
                nc.sync.dma_start(mA_sb[:, :, 768:TRI], mA_d[:, :, 768:TRI])
                qtc(1152, 2112)
                nc.sync.dma_start(mB_sb[:, :, 768:TRI], mB_d[:, :, 768:TRI])
                nc.sync.dma_start(m24_sb[:, 768:TRI], m24_d[:, 768:TRI])
                qtc(2112, S)
            else:
                for c in range(5):
                    c0, c1 = c * 634, min((c + 1) * 634, S)
                    ktc(c0, c1)
                    v1c(c * 5, min((c + 1) * 5, NT))
                for c in range(4):
                    qtc(c * 792, (c + 1) * 792)
            nc.sync.dma_start(v2[:, :, :], v2_d[h])
            nc.sync.dma_start(v3[:, :, :], v3_d[h])

        # ---- per-group mask info: (sbuf_ap_fn, part, jslice) or None
        def mask_for(gi, q0, qw):
            if gi == 2:    # tile 4 in slot j=0 (j=1 mask is zeros)
                return (m4_sb[:, :, q0:q0 + qw], 128, 0, 2)
            if gi == 10:   # tiles 20,21 both masked
                return (mA_sb[:, :, q0:q0 + qw], 128, 0, 2)
            if gi == 11:   # tiles 22,23
                return (mB_sb[:, :, q0:q0 + qw], 128, 0, 2)
            if gi == 12:   # half tile 24
                return (m24_sb[:, q0:q0 + qw], 64, 0, 1)
            return None

        items = []
        for h in range(HPC):
            kt, qt, v1 = kt_sb[h], qt_sb[h], v1_sb[h]

            # ===== seg1 items =====
            head_items = []
            for qbi, (q0, qw) in enumerate(QBLOCKS):
                subs = _subs(qw)
                blk = {}

                def qk1(st, kt=kt, qt=qt, q0=q0, qw=qw, tiles=None):
                    for j, t in enumerate(tiles):
                        kw = min(128, TRIR - t * 128)
                        nc.tensor.matmul(
                            st[0:kw, j, 0:qw],
                            lhsT=kt[:, t * 128: t * 128 + kw],
                            rhs=qt[:, q0:q0 + qw],
                            start=True, stop=True)

                def ex_act(st, pt, q0=q0, qw=qw, gi=None, tiles=None):
                    mk = mask_for(gi, q0, qw)
                    if mk is not None:
                        # fused DVE Schraudolph exp+mask: one stt op
                        m_ap, part, j0, jn = mk
                        nc.vector.scalar_tensor_tensor(
                            pt.bitcast(I16)[0:part, j0:j0 + jn, 0:qw],
                            st[0:part, j0:j0 + jn, 0:qw],
                            SBIAS + SCHC, m_ap, ADD, ADD)
                        return
                    ntl = len(tiles)
                    nc.scalar.activation(
                        pt[:, 0:ntl, 0:qw], st[:, 0:ntl, 0:qw],
                        EXP, scale=1.0 / KEXP)

                def ex_dve(st, pt, qw=qw, tiles=None):
                    ntl = len(tiles)
                    nc.vector.tensor_scalar(
                        pt.bitcast(I16)[:, 0:ntl, 0:qw], st[:, 0:ntl, 0:qw],
                        SBIAS + SCHC, None, ADD)

                def pv1(pt, h=h, v1=v1, q0=q0, qw=qw, subs=subs, blk=blk,
                        qbi=qbi, tiles=None):
                    if "acc" not in blk:
                        blk["acc"] = accp.tile([128, 512], F32, tag="acc",
                                               name="acc")
                    acc = blk["acc"]
                    for j, t in enumerate(tiles):
                        kw = min(128, TRIR - t * 128)
                        for si, (qs0, qsw) in enumerate(subs):
                            nc.tensor.matmul(
                                acc[0:qsw, si * 129: si * 129 + 129],
                                lhsT=pt[0:kw, j, qs0:qs0 + qsw],
                                rhs=v1[0:kw, t, :],
                                start=(t == 0 and si == 0),
                                stop=(t == NT - 1))
                    if NT - 1 in tiles:
                        # qblock done: stage unnormalized acc (+den) to SBUF
                        # bf16 and DMA out; host divides. The global last
                        # qblock stages per-sub so the tail chain is short.
                        stg = stgp.tile([128, 512], BF16, tag="stg", name="stg")
                        nc.vector.tensor_scalar(
                            stg[:, 0:387], acc[:, 0:387], 0.0, None, BYP)
                        nc.sync.dma_start(o1_d[h, qbi], stg[:, 0:387])

                for gi, tiles in enumerate(GROUPS):
                    use_dve = gi in DVE_GROUPS
                    head_items.append((
                        (lambda st, f=qk1, tl=tiles: f(st, tiles=tl)),
                        (lambda st, pt, f=ex_dve, tl=tiles: f(st, pt, tiles=tl))
                        if use_dve else
                        (lambda st, pt, f=ex_act, g=gi, tl=tiles:
                         f(st, pt, gi=g, tiles=tl)),
                        (lambda pt, f=pv1, tl=tiles: f(pt, tiles=tl)),
                    ))

            # ===== seg2 items: per-cond ref self-attention =====
            seg23_items = []
            for c in range(NCOND):
                b0 = TRI + 256 * c

                def qk2(st, kt=kt, qt=qt, b0=b0):
                    for j in range(2):
                        nc.tensor.matmul(
                            st[:, j, 0:256],
                            lhsT=kt[:, b0 + j * 128: b0 + (j + 1) * 128],
                            rhs=qt[:, b0: b0 + 256],
                            start=True, stop=True)

                def ex2(st, pt):
                    nc.scalar.activation(pt[:, 0:2, 0:256], st[:, 0:2, 0:256],
                                         EXP, scale=1.0 / KEXP)

                def pv2(pt, h=h, v2=v2_sb[h], c=c):
                    acc = accp.tile([128, 512], F32, tag="acc", name="acc")
                    for j in range(2):
                        for si in range(2):
                            nc.tensor.matmul(
                                acc[0:128, si * 129: si * 129 + 129],
                                lhsT=pt[:, j, si * 128:(si + 1) * 128],
                                rhs=v2[:, 2 * c + j, :],
                                start=(j == 0 and si == 0), stop=(j == 1))
                    stg = stgp.tile([128, 512], BF16, tag="stg", name="stg")
                    nc.vector.tensor_scalar(
                        stg[:, 0:258], acc[:, 0:258], 0.0, None, BYP)
                    nc.sync.dma_start(o2_d[h, c], stg[:, 0:258])

                seg23_items.append((qk2, ex2, pv2))

            # ===== seg3 item: router queries =====
            def qk3(st, kt=kt, qt=qt):
                for i in range(16):
                    nc.tensor.matmul(
                        st[:, 0, i * 32:(i + 1) * 32],
                        lhsT=kt[:, TRE + i * 128: TRE + (i + 1) * 128],
                        rhs=qt[:, TRIR: TRIR + 32],
                        start=True, stop=True)
                nc.tensor.matmul(
                    st[0:32, 1, 0:32],
                    lhsT=kt[:, TRIR: TRIR + 32],
                    rhs=qt[:, TRIR: TRIR + 32],
                    start=True, stop=True)

            def ex3(st, pt):
                nc.scalar.activation(pt[:, 0, 0:512], st[:, 0, 0:512],
                                     EXP, scale=1.0 / KEXP)
                nc.scalar.activation(pt[0:32, 1, 0:32], st[0:32, 1, 0:32],
                                     EXP, scale=1.0 / KEXP)

            def pv3(pt, h=h, v3=v3_sb[h]):
                acc = accp.tile([128, 512], F32, tag="acc", name="acc")
                for i in range(16):
                    nc.tensor.matmul(
                        acc[0:32, 0:129],
                        lhsT=pt[:, 0, i * 32:(i + 1) * 32],
                        rhs=v3[:, i, :],
                        start=(i == 0), stop=False)
                nc.tensor.matmul(
                    acc[0:32, 0:129],
                    lhsT=pt[0:32, 1, 0:32],
                    rhs=v3[0:32, 16, :],
                    start=False, stop=True)
                stg = stgp.tile([128, 512], BF16, tag="stg", name="stg")
                nc.vector.tensor_scalar(
                    stg[0:32, 0:129], acc[0:32, 0:129], 0.0, None, BYP)
                nc.sync.dma_start(o3_d[h], stg[0:32, 0:129])

            seg23_items.append((qk3, ex3, pv3))
            # splice seg2/3 mid-stream so their small exp bursts don't pile
            # at the head boundary
            for i, it in enumerate(seg23_items):
                head_items.insert(40 + i * 6, it)
            items.extend(head_items)

        # ---- global 2-deep software pipeline
        pending = []
        for (fqk, fex, fpv) in items:
            st = stp.tile([128, 2, 512], F32, tag="st", name="st")
            fqk(st)
            while len(pending) >= 5:
                pending.pop(0)()
            pt = ptp.tile([128, 2, 512], BF16, tag="pt", name="pt")
            fex(st, pt)
            pending.append(lambda f=fpv, p=pt: f(p))
        while pending:
            pending.pop(0)()

    nc.compile()
    return nc


_NC_CACHE = None


def _get_nc():
    global _NC_CACHE
    if _NC_CACHE is None:
        _NC_CACHE = build_nc()
    return _NC_CACHE


def make_in_maps(query, key, value, ref_mask, routing_map):
    BF = ml_dtypes.bfloat16
    q = np.asarray(query, np.float32)[0] * (KEXP / math.sqrt(D))  # [24,S,128]
    k = np.asarray(key, np.float32)[0]
    v = np.asarray(value, np.float32)[0]
    qt = np.ascontiguousarray(q.transpose(0, 2, 1)).astype(BF)    # [24,128,S]
    kt = np.ascontiguousarray(k.transpose(0, 2, 1)).astype(BF)    # [24,128,S]

    vv = np.zeros((H, NT * 128, 129), np.float32)
    vv[:, :TRIR, :128] = v[:, :TRIR]
    vv[:, :TRIR, 128] = 1.0
    vv = vv.astype(BF)
    v1 = np.ascontiguousarray(
        vv.reshape(H, NT, 128, 129).transpose(0, 2, 1, 3))        # [24,128,NT,129]
    vv2 = np.zeros((H, 512, 129), np.float32)
    vv2[:, :, :128] = v[:, TRI:TRIR]
    vv2[:, :, 128] = 1.0
    v2 = np.ascontiguousarray(
        vv2.astype(BF).reshape(H, 4, 128, 129).transpose(0, 2, 1, 3))
    v3 = np.zeros((H, 17, 128, 129), np.float32)
    v3[:, 0:16, :, :128] = v[:, TRE:TRI].reshape(H, 16, 128, 128)
    v3[:, 0:16, :, 128] = 1.0
    v3[:, 16, 0:32, :128] = v[:, TRIR:TRIR + 32]
    v3[:, 16, 0:32, 128] = 1.0
    v3 = np.ascontiguousarray(v3.astype(BF).transpose(0, 2, 1, 3))

    rm = np.asarray(ref_mask, np.float32)[0]                      # [512, 2624]
    rt = np.asarray(routing_map, np.float32)[0]                   # [2, 2048]
    base = (rm - 1.0) * 100.0 + REF_SHIFT
    base = base.copy()
    base[:, TRE:TRI] += (np.repeat(rt, REF // NCOND, axis=0) - 1.0) * 100.0
    base = np.maximum(base * KEXP, MASK_CLAMP)                    # [512, 2624]
    mredux = np.maximum(
        (np.repeat(rt, REDUX // NCOND, axis=0) - 1.0) * 100.0 * KEXP,
        MASK_CLAMP)                                               # [64, 2048]

    m4 = np.zeros((128, 2, TRI), np.float32)
    m4[0:64, 0, TRE:TRI] = mredux
    mA = np.zeros((2, 128, TRI), np.float32)                      # tiles 20,21
    mA[0, 64:128] = base[0:64]
    mA[1] = base[64:192]
    mB = np.zeros((2, 128, TRI), np.float32)                      # tiles 22,23
    mB[0] = base[192:320]
    mB[1] = base[320:448]
    m24 = base[448:512]                                           # [64, 2624]

    in_maps = []
    for c in range(8):
        hs = slice(HPC * c, HPC * (c + 1))
        in_maps.append({
            "qt": np.ascontiguousarray(qt[hs]),
            "kt": np.ascontiguousarray(kt[hs]),
            "v1": np.ascontiguousarray(v1[hs]),
            "v2": np.ascontiguousarray(v2[hs]),
            "v3": np.ascontiguousarray(v3[hs]),
            "m4": np.ascontiguousarray(m4).astype(BF),
            "mA": np.ascontiguousarray(mA.transpose(1, 0, 2)).astype(BF),
            "mB": np.ascontiguousarray(mB.transpose(1, 0, 2)).astype(BF),
            "m24": m24.astype(BF),
        })
    return in_maps


def _assemble(res):
    """Host-side gather + normalize: bf16 accumulators -> f32 output."""
    full = np.zeros((H, S, D), np.float32)
    for c in range(8):
        r = res.results[c]
        o1 = np.asarray(r["o1"], np.float32)   # [3,7,128,387]
        o2 = np.asarray(r["o2"], np.float32)   # [3,2,128,258]
        o3 = np.asarray(r["o3"], np.float32)   # [3,32,129]
        for hh in range(HPC):
            h = c * HPC + hh
            for qbi, (q0, qw) in enumerate(QBLOCKS):
                for si, (qs0, qsw) in enumerate(_subs(qw)):
                    blkv = o1[hh, qbi, 0:qsw, si * 129: si * 129 + 128]
                    den = o1[hh, qbi, 0:qsw, si * 129 + 128: si * 129 + 129]
                    full[h, q0 + qs0: q0 + qs0 + qsw] = blkv / den
            for cc in range(NCOND):
                b0 = TRI + 256 * cc
                for si in range(2):
                    blkv = o2[hh, cc, :, si * 129: si * 129 + 128]
                    den = o2[hh, cc, :, si * 129 + 128: si * 129 + 129]
                    full[h, b0 + si * 128: b0 + (si + 1) * 128] = blkv / den
            full[h, TRIR:] = o3[hh, :, 0:128] / o3[hh, :, 128:129]
    return np.ascontiguousarray(full[None].astype(np.float32))


def kernel(query, key, value, ref_mask, routing_map, **_ignored):
    import jax
    if not any(d.platform == "axon" for d in jax.devices()):
        jax.config.update("jax_platforms", "axon,cpu")
    nc = _get_nc()
    in_maps = make_in_maps(query, key, value, ref_mask, routing_map)
    res = run_bass_kernel_spmd(nc, in_maps, core_ids=list(range(8)))
    return _assemble(res)
